# revision 20
# baseline (speedup 1.0000x reference)
"""Trainium2 Bass kernel: separable box filter (radius 4) on (8,3,1024,1024) fp32.

v6: DVE/PE hybrid, fp16 end-to-end (host casts f32<->f16; rel-err budget
2e-2 vs ~7e-4 worst-case fp16 error).

 - W pass (7 tiles/slice) on DVE: tensor_tensor_scan running 9-tap box sum
   (state is fp32 internally).  This is the kernel's hard floor: the scan
   runs at 2.08 ns/elem with no fast modes (measured).
 - Two tiles per slice (t=4,5) skip the scan: the PE computes their 9x9 box
   directly as 9 W-shifted accumulating band matmuls per 512-wide half,
   emitted as one contiguous stretch so the PE p-state ramps.
 - H pass: banded weights W[k, m] = 1 iff m <= k <= m+8.  Edge tiles use
   K-sliced matmuls instead of zeroed halo partitions (tile 0 loads rows
   0..123 unshifted with its own band wp0; tile 8 contracts over K=68), so
   input DMAs carry no waits and the schedule keeps them first.
 - ACT drains PSUM f32 -> SBUF fp16; scan-PSUM ring depth 3 keeps the tail
   matmul->ACT chain from serializing; direct-PSUM is a half-width ring so
   drains overlap the stretch.
"""

import numpy as np

H = 1024
W = 1024
R = 4
D = 2 * R + 1
N_CORES = 8
SLICES_PER_CORE = 3
TILE_OUT = 120
N_TILES = 9
P_W = D + W + R  # 9 left zeros + 1024 data + 4 right zeros
S_W = W + R

DIRECT = (4, 5)
SCAN_TILES = [0, 1, 2, 3, 6, 7, 8]

_COMPILED = {}


def _band_weights():
    """Standard band: lhsT[k, m] = 1 iff m <= k <= m+8 (tile rows start at
    global row 120t-4)."""
    k = np.arange(128)[:, None]
    m = np.arange(TILE_OUT)[None, :]
    return ((m <= k) & (k <= m + 2 * R)).astype(np.float16)


def _band_weights0():
    """Tile-0 band for unshifted load (partition p = global row p):
    lhsT[k, m] = 1 iff m-4 <= k <= m+4 (left truncation via k >= 0)."""
    k = np.arange(128)[:, None]
    m = np.arange(TILE_OUT)[None, :]
    return ((m - R <= k) & (k <= m + R) & (k < 124)).astype(np.float16)


def _build():
    from concourse import bacc, mybir
    from concourse.tile import TileContext

    f16 = mybir.dt.float16
    f32 = mybir.dt.float32
    nc = bacc.Bacc("TRN2", target_bir_lowering=False, debug=False,
                   num_devices=N_CORES)

    x = nc.dram_tensor("x", (SLICES_PER_CORE, H, W), f16,
                       kind="ExternalInput").ap()
    wp = nc.dram_tensor("wp", (128, TILE_OUT), f16, kind="ExternalInput").ap()
    wp0 = nc.dram_tensor("wp0", (128, TILE_OUT), f16,
                         kind="ExternalInput").ap()
    out = nc.dram_tensor("out", (SLICES_PER_CORE, H, W), f16,
                         kind="ExternalOutput").ap()

    add = mybir.AluOpType.add
    sub = mybir.AluOpType.subtract
    act_copy = mybir.ActivationFunctionType.Copy

    from concourse.ap import AP

    xh = x.tensor
    oh = out.tensor

    def src_windows(s, t0, nt):
        off = s * H * W + (TILE_OUT * t0 - R) * W
        return AP(xh, off, [[W, 128], [TILE_OUT * W, nt], [1, W]])

    def dst_rows(s, t0, nt):
        off = s * H * W + TILE_OUT * t0 * W
        return AP(oh, off, [[W, TILE_OUT], [TILE_OUT * W, nt], [1, W]])

    def in_dma(xc, s, t):
        if t == 0:
            # unshifted: partition p = global row p, rows 0..123
            nc.sync.dma_start(xc[0:124, D:D + W], x[s, 0:124, :])
        elif t == 8:
            nc.sync.dma_start(xc[0:68, D:D + W], x[s, 8 * TILE_OUT - R:H, :])
        else:
            nc.sync.dma_start(xc[:, D:D + W], src_windows(s, t, 1)[:, 0, :])

    def pad_cols(xc):
        nc.gpsimd.memset(xc[:, 0:D], 0.0)
        nc.gpsimd.memset(xc[:, D + W:P_W], 0.0)

    def band_for(t):
        if t == 0:
            return 124  # wp0, K=124
        if t == 8:
            return 68  # wp, K=68
        return 128

    with TileContext(nc) as tc:
        with tc.tile_pool(name="wts", bufs=1) as wpool, \
             tc.tile_pool(name="xp", bufs=1) as xpool, \
             tc.tile_pool(name="sc", bufs=10) as spool, \
             tc.tile_pool(name="outp", bufs=8) as opool, \
             tc.tile_pool(name="ps", bufs=3, space="PSUM") as pspool, \
             tc.tile_pool(name="psd", bufs=2, space="PSUM") as dpool:
            xbufs = []
            for t in range(N_TILES):
                xb = xpool.tile([128, P_W], f16, tag=f"xc{t}")
                xbufs.append(xb)

            wp_t = wpool.tile([128, TILE_OUT], f16)
            nc.scalar.dma_start(wp_t[:], wp[:])
            wp0_t = wpool.tile([128, TILE_OUT], f16)
            nc.scalar.dma_start(wp0_t[:], wp0[:])

            for s in range(SLICES_PER_CORE):
                # scan tile 0's input first: wait-free DMA, shortest fill
                in_dma(xbufs[0], s, 0)
                if s == 0:
                    pad_cols(xbufs[0])

                # ---- direct (PE) pair: 9 shifted band matmuls per half ----
                for t in DIRECT:
                    in_dma(xbufs[t], s, t)
                    if s == 0:
                        pad_cols(xbufs[t])
                ocd = opool.tile([TILE_OUT, 2, W], f16, tag="ocd")
                for di, t in enumerate(DIRECT):
                    xc = xbufs[t]
                    for hf in range(2):
                        w0 = 512 * hf
                        psd = dpool.tile([TILE_OUT, 512], f32)
                        for j in range(D):
                            nc.tensor.matmul(
                                psd[:], wp_t[:],
                                xc[:, 5 + w0 + j:5 + w0 + j + 512],
                                start=(j == 0), stop=(j == D - 1))
                        nc.scalar.activation(ocd[:, di, w0:w0 + 512],
                                             psd[:], act_copy)
                nc.scalar.dma_start(dst_rows(s, DIRECT[0], 2), ocd[:, 0:2, :])

                # ---- scan tiles on DVE ----
                # last slice: interleave so the final ACT drains spread out
                # (tail shrink); pairs stay adjacent for the output DMA.
                order = [0, 1, 6, 7, 2, 3, 8] if s == 2 else SCAN_TILES
                for idx, t in enumerate(order):
                    xc = xbufs[t]
                    if t != 0:
                        in_dma(xc, s, t)
                        if s == 0:
                            pad_cols(xc)

                    kp = band_for(t)
                    if idx % 2 == 0:
                        oc = opool.tile([TILE_OUT, 2, W], f16, tag="oc")
                    oi = idx % 2
                    m = min(TILE_OUT, H - TILE_OUT * t)
                    st = spool.tile([128, S_W], f16)
                    nc.vector.tensor_tensor_scan(
                        st[0:kp, :], xc[0:kp, D:P_W], xc[0:kp, 0:S_W],
                        0.0, add, sub)
                    lhs = wp0_t if t == 0 else wp_t
                    ps = pspool.tile([TILE_OUT, 2 * 512], f32)
                    for hf in range(2):
                        w0 = 512 * hf
                        nc.tensor.matmul(ps[0:m, w0:w0 + 512],
                                         lhs[0:kp, 0:m],
                                         st[0:kp, w0 + R:w0 + R + 512],
                                         start=True, stop=True)
                    if s == 2 and t == 3:
                        # defer this drain + its pair DMA to after the last
                        # scan: the drain runs on the then-idle DVE without
                        # blocking scan (2,8) behind it on the DVE queue
                        deferred = (oc, ps, m, oi)
                    elif s == 2 and t == 8:
                        # tail drains on the DVE (idle after its last scan),
                        # in parallel with ACT's remaining copies; both
                        # output DMAs go last on the scalar queue so their
                        # cross-engine waits block nothing
                        doc, dps, dm, doi = deferred
                        nc.vector.tensor_copy(doc[0:dm, doi, :],
                                              dps[0:dm, :])
                        nc.vector.tensor_copy(oc[0:m, oi, :], ps[0:m, :])
                        nc.scalar.dma_start(dst_rows(s, 2, 2),
                                            doc[:, 0:2, :])
                        nc.scalar.dma_start(out[s, 8 * TILE_OUT:H, :],
                                            oc[0:64, 0, :])
                    else:
                        nc.scalar.activation(oc[0:m, oi, :], ps[0:m, :],
                                             act_copy)
                        if t == 8:
                            nc.scalar.dma_start(out[s, 8 * TILE_OUT:H, :],
                                                oc[0:64, 0, :])
                        elif oi == 1:
                            nc.scalar.dma_start(dst_rows(s, t - 1, 2),
                                                oc[:, 0:2, :])

    nc.compile()
    return nc


def _get_nc():
    if "nc" not in _COMPILED:
        _COMPILED["nc"] = _build()
    return _COMPILED["nc"]


def _in_maps(x: np.ndarray):
    xf = np.ascontiguousarray(np.asarray(x, dtype=np.float16)).reshape(
        N_CORES * SLICES_PER_CORE, H, W)
    wp_np = _band_weights()
    wp0_np = _band_weights0()
    return [{
        "x": xf[c * SLICES_PER_CORE:(c + 1) * SLICES_PER_CORE],
        "wp": wp_np,
        "wp0": wp0_np,
    } for c in range(N_CORES)]


def kernel(x: np.ndarray) -> np.ndarray:
    from concourse.bass_utils import run_bass_kernel_spmd

    nc = _get_nc()
    res = run_bass_kernel_spmd(nc, _in_maps(x), core_ids=list(range(N_CORES)))
    outs = [res.results[c]["out"] for c in range(N_CORES)]
    return np.concatenate(outs, axis=0).reshape(8, 3, H, W).astype(np.float32)
